# revision 54
# baseline (speedup 1.0000x reference)
"""Causal self-attention on 8 TRN2 NeuronCores (Bass/Tile, SPMD).

Problem: B=4, T=2048, C=1024, NH=16, HS=64.
  qkv = x @ W_attn + b_attn; causal softmax attention per head; y @ W_proj + b_proj.

Sharding: core = (batch b, class) with b = core//2, class = core%2.
Each core computes attention + output projection for 1024 of its batch's
queries: the even 128-token tiles (class E) or the odd tiles (class O).
Interleaving at tile granularity balances causal work exactly: both classes
cover 72 (128x128) score tiles per head (vs 96 for block pairing).

SPMD uniformity: all 8 cores run the same instruction stream. Per-core data
(the x permutation + kill-mask tiles) absorbs the class differences:
 - x arrives host-transposed (x^T, [C, T]) and host-permuted so that permuted
   token-tile i holds: own tiles ascending (slots 0-7), then the other class's
   tiles ascending (slots 8-15).
 - stream slot i covers the suffix window of SLOTN[i] own-query tiles; slots
   0-7 front tile is the causal diagonal (static triangular mask), slots 8-15
   front tile is either fully acausal (class E: data-zero kill mask) or fully
   causal (class O: ones).

Per pack p (heads 2p,2p+1), attention runs in two phases so the 4 y-PSUM
tiles fit in 2 banks at a time: phase A computes the 512-wide front subslot
of slots {0,1,2,3,8,9,10,11} and accumulates their qb0 columns; the exp'd
scores are retained in SBUF so phase B can consume their qb1 slivers without
recomputation, alongside the tail/pure qb1 subslots.

Matmuls in bf16 (1 col/cycle); the per-head S matmuls (K=64) are row-tiled
via base_partition and execute concurrently in the PE array. Softmax skips
max-subtraction (logits ~N(0,0.4)); row sums come from a ones-column in V'.
"""

import numpy as np
from contextlib import ExitStack

B, T, C = 4, 2048, 1024
NH, HS = 16, 64
P = 128
NCORES = 8
VPW = NH * (HS + 1)   # 1040: V' columns (per-head 64 V cols + ones col)

# tiles covered by stream slot i (suffix window of own-query axis)
SLOTN = [8, 7, 6, 5, 4, 3, 2, 1, 8, 7, 6, 5, 4, 3, 2, 1]
PROC_A = [0, 8, 1, 9, 2, 10, 3, 11]      # front subslots (512 wide)
# phase B: tails of A-slots (qb1 remainder), pures (windows inside qb1)
PROC_B = [(0, 't'), (8, 't'), (4, 'p'), (12, 'p'),
          (1, 't'), (9, 't'), (5, 'p'), (13, 'p'),
          (2, 't'), (10, 't'), (6, 'p'), (14, 'p'),
          (3, 't'), (11, 't'), (7, 'p'), (15, 'p')]


def _build_program():
    import concourse.bacc as bacc
    import concourse.tile as tile
    from concourse import mybir
    from concourse.mybir import ActivationFunctionType as AFT

    f32 = mybir.dt.float32
    bf16 = mybir.dt.bfloat16

    nc = bacc.Bacc("TRN2", target_bir_lowering=False, debug=False,
                   num_devices=NCORES)

    xkd = nc.dram_tensor("xkT", [C, T], bf16, kind="ExternalInput").ap()
    wqk = nc.dram_tensor("wqk", [C, 2 * C], bf16, kind="ExternalInput").ap()
    bqk = nc.dram_tensor("bqk", [P, 16], f32, kind="ExternalInput").ap()
    wvp = nc.dram_tensor("wvp", [C, VPW], bf16, kind="ExternalInput").ap()
    bvp = nc.dram_tensor("bvp", [P, VPW], f32, kind="ExternalInput").ap()
    wpj = nc.dram_tensor("wproj", [C, C], bf16, kind="ExternalInput").ap()
    bpj = nc.dram_tensor("bproj", [P, C], f32, kind="ExternalInput").ap()
    bzd = nc.dram_tensor("bz", [P, 1], f32, kind="ExternalInput").ap()
    trid = nc.dram_tensor("tri", [P, P], bf16, kind="ExternalInput").ap()
    kmd = nc.dram_tensor("kmask", [8, P, P], bf16, kind="ExternalInput").ap()
    outd = nc.dram_tensor("out", [1024, C], f32, kind="ExternalOutput").ap()

    with tile.TileContext(nc) as tc:
        with ExitStack() as octx:
            yt_pool = octx.enter_context(tc.tile_pool(name="yt", bufs=8))
            yT = [yt_pool.tile([P, 1024], bf16, tag="yt", name=f"yT{i}")
                  for i in range(8)]

            with ExitStack() as ctx:
                # ---- pools ---------------------------------------------
                xk_pool = ctx.enter_context(tc.tile_pool(name="xk", bufs=16))
                vs_pool = ctx.enter_context(tc.tile_pool(name="vs", bufs=64))
                kt_pool = ctx.enter_context(tc.tile_pool(name="ktp", bufs=2))
                qt_pool = ctx.enter_context(tc.tile_pool(name="qtp", bufs=2))
                ptA_pool = ctx.enter_context(tc.tile_pool(name="ptA", bufs=8))
                ptB_pool = ctx.enter_context(tc.tile_pool(name="ptB", bufs=3))
                sm_pool = ctx.enter_context(tc.tile_pool(name="sm", bufs=3))
                rec_pool = ctx.enter_context(tc.tile_pool(name="rec", bufs=8))
                # PSUM: span 2x2 banks + y 2x1 + qkv-acc 2x1 = 8 banks
                span_p = ctx.enter_context(tc.tile_pool(name="span", bufs=2, space="PSUM"))
                yp_p = ctx.enter_context(tc.tile_pool(name="yp", bufs=2, space="PSUM"))
                sh_p = ctx.enter_context(tc.tile_pool(name="shp", bufs=2, space="PSUM"))

                # ---- input DMAs (order = arrival priority) -------------
                # K-side weights + x^T first (pack-0 kt gates on them).
                wq_pool = ctx.enter_context(tc.tile_pool(name="wqk", bufs=8))
                wqk_sb = [wq_pool.tile([P, 2 * C], bf16, tag="wqk", name=f"wqk{i}")
                          for i in range(8)]
                xk = [[None] * 8 for _ in range(2)]
                for c in range(8):
                    nc.sync.dma_start(wqk_sb[c][:], wqk[c * P:(c + 1) * P, :])
                for h in range(2):
                    for c in range(8):
                        t = xk_pool.tile([P, 1024], bf16, tag="xk", name=f"xk{h}_{c}")
                        nc.sync.dma_start(t[:], xkd[c * P:(c + 1) * P, h * 1024:(h + 1) * 1024])
                        xk[h][c] = t

                bq_pool = ctx.enter_context(tc.tile_pool(name="bq", bufs=1))
                bqk_sb = bq_pool.tile([P, 16], f32, tag="bqk")
                nc.sync.dma_start(bqk_sb[:], bqk)
                bz_sb = bq_pool.tile([P, 1], f32, tag="bz")
                nc.sync.dma_start(bz_sb[:], bzd)
                mpool = ctx.enter_context(tc.tile_pool(name="masks", bufs=9))
                tri_sb = mpool.tile([P, P], bf16, tag="tri")
                nc.sync.dma_start(tri_sb[:], trid)
                km_sb = [mpool.tile([P, P], bf16, tag="km", name=f"km{i}")
                         for i in range(8)]
                for i in range(8):
                    nc.sync.dma_start(km_sb[i][:], kmd[i])

                wv_pool = ctx.enter_context(tc.tile_pool(name="wvp", bufs=8))
                wvp_sb = [wv_pool.tile([P, VPW], bf16, tag="wvp", name=f"wvp{i}")
                          for i in range(8)]
                for c in range(8):
                    nc.sync.dma_start(wvp_sb[c][:], wvp[c * P:(c + 1) * P, :])
                bvp_sb = bq_pool.tile([P, VPW], f32, tag="bvp")
                nc.sync.dma_start(bvp_sb[:], bvp)

                # ---- qkv emission units (software pipelining) ----------
                v_sb = [[None] * 16 for _ in range(4)]
                kt_tiles = {}
                qt_tiles = {}

                def unit_v(g, s):
                    def emit():
                        n0 = 260 * g
                        h, tt = s // 8, s % 8
                        acc = sh_p.tile([P, 512], f32, tag="shp")
                        for c in range(8):
                            nc.tensor.matmul(acc[:, 0:260],
                                             xk[h][c][:, tt * P:(tt + 1) * P],
                                             wvp_sb[c][:, n0:n0 + 260],
                                             start=(c == 0), stop=(c == 7))
                        vt = vs_pool.tile([P, 260], bf16, tag="vs",
                                          name=f"v{g}_{s}")
                        nc.vector.tensor_add(vt[:], acc[:, 0:260],
                                             bvp_sb[:, n0:n0 + 260])
                        v_sb[g][s] = vt
                    return emit

                def unit_k(p, ts):
                    def emit():
                        if p not in kt_tiles:
                            kt_tiles[p] = kt_pool.tile([P, T], bf16, tag="kt",
                                                       name=f"kt{p}")
                        kt = kt_tiles[p]
                        h, w0 = ts // 2, (ts % 2) * 512
                        acc = sh_p.tile([P, 512], f32, tag="shp")
                        for c in range(8):
                            nc.tensor.matmul(acc[:],
                                             wqk_sb[c][:, (8 + p) * P:(9 + p) * P],
                                             xk[h][c][:, w0:w0 + 512],
                                             start=(c == 0), stop=(c == 7))
                        nc.vector.tensor_scalar_add(kt[:, ts * 512:(ts + 1) * 512],
                                                    acc[:], bqk_sb[:, 8 + p:9 + p])
                    return emit

                def unit_q(p, qi):
                    def emit():
                        if p not in qt_tiles:
                            qt_tiles[p] = qt_pool.tile([P, 1024], bf16, tag="qt",
                                                       name=f"qt{p}")
                        qt = qt_tiles[p]
                        acc = sh_p.tile([P, 512], f32, tag="shp")
                        for c in range(8):
                            nc.tensor.matmul(acc[:],
                                             wqk_sb[c][:, p * P:(p + 1) * P],
                                             xk[0][c][:, qi * 512:(qi + 1) * 512],
                                             start=(c == 0), stop=(c == 7))
                        nc.vector.tensor_scalar_add(qt[:, qi * 512:(qi + 1) * 512],
                                                    acc[:], bqk_sb[:, p:p + 1])
                    return emit

                def qkv_units(p):
                    units = [unit_k(p, ts) for ts in range(4)]
                    units += [unit_q(p, qi) for qi in range(2)]
                    if p % 2 == 0:
                        units += [unit_v(p // 2, s) for s in range(16)]
                    return units

                def norm_units(p, recs):
                    # recs: 4 x [1,512] f32 reciprocal rows (qb0h0,qb0h1,qb1h0,qb1h1)
                    units = []
                    for qb in range(2):
                        for hh in range(2):
                            def u_norm(qb=qb, hh=hh):
                                qsl = slice(qb * 512, qb * 512 + 512)
                                rec = recs[qb * 2 + hh]
                                rcst = sm_pool.tile([1, 512], bf16, tag="rcst")
                                nc.vector.tensor_copy(rcst[:], rec[:])
                                bcs = sm_pool.tile([P, 512], bf16, tag="bcs")
                                nc.gpsimd.partition_broadcast(bcs[:], rcst[:],
                                                              channels=P)
                                nc.vector.tensor_mul(
                                    yT[p][hh * 64:(hh + 1) * 64, qsl],
                                    yT[p][hh * 64:(hh + 1) * 64, qsl],
                                    bcs[hh * 64:(hh + 1) * 64, :])
                            units.append(u_norm)
                    return units

                def apply_mask(pt, s, n):
                    """mask front 128 cols of both heads' windows (width n)."""
                    m = tri_sb if s < 8 else km_sb[s - 8]
                    for hh in range(2):
                        sl = slice(hh * n, hh * n + P)
                        nc.vector.tensor_mul(pt[:, sl], pt[:, sl], m[:])

                # ---- main pipeline over head-packs ---------------------
                for u in qkv_units(0):      # prologue
                    u()

                pend_norm = []
                for p in range(8):
                    pend = qkv_units(p + 1) if p < 7 else []
                    pend = pend[:6] + pend_norm + pend[6:]
                    total_u, emitted, si = len(pend), 0, 0
                    n_points = len(PROC_A) + len(PROC_B)
                    kt, qt = kt_tiles[p], qt_tiles[p]
                    g, off = p // 2, (p % 2) * 130
                    recs = [rec_pool.tile([1, 512], f32, tag="rec", name=f"rec{i}")
                            for i in range(4)]

                    def pump():
                        nonlocal emitted, si
                        si += 1
                        want = total_u * si // n_points
                        while emitted < want:
                            pend.pop(0)()
                            emitted += 1

                    ptA = {}
                    # ---------- phase A: front subslots, qb0 ------------
                    ya = [yp_p.tile([HS + 1, 512], f32, tag="yp", name=f"ya{hh}")
                          for hh in range(2)]
                    for ui, s in enumerate(PROC_A):
                        ws = (8 - SLOTN[s]) * P   # window start (own-q cols)
                        ksl = slice(s * P, (s + 1) * P)
                        span = span_p.tile([P, 1024], f32, tag="span")
                        nc.tensor.matmul(span[:, 0:512], kt[0:64, ksl],
                                         qt[0:64, ws:ws + 512], start=True, stop=True)
                        nc.tensor.matmul(span[:, 512:1024], kt[64:128, ksl],
                                         qt[64:128, ws:ws + 512], start=True, stop=True)
                        pt = ptA_pool.tile([P, 1024], bf16, tag="ptA",
                                           name=f"ptA{ui}")
                        nc.scalar.activation(pt[:], span[:], AFT.Exp,
                                             bias=bz_sb[:], scale=0.125)
                        apply_mask(pt, s, 512)
                        ptA[s] = pt
                        # qb0 part: window cols [ws:512) -> rel [0 : 512-ws)
                        n0 = 512 - ws
                        for hh in range(2):
                            nc.tensor.matmul(
                                ya[hh][:, ws:512],
                                v_sb[g][s][:, off + hh * 65:off + hh * 65 + 65],
                                pt[:, hh * 512:hh * 512 + n0],
                                start=(ui == 0), stop=(ui == len(PROC_A) - 1))
                        pump()
                    # evict ya -> yT cols [0:512], reciprocal of sums row
                    for hh, yy in ((0, ya[0]), (1, ya[1])):
                        with nc.allow_low_precision(reason="softmax recip"):
                            nc.vector.reciprocal(recs[hh][:], yy[64:65, :])
                        nc.vector.tensor_copy(yT[p][hh * 64:(hh + 1) * 64, 0:512],
                                              yy[0:64, :])

                    # ---------- phase B: qb1 ----------------------------
                    yb = [yp_p.tile([HS + 1, 512], f32, tag="yp", name=f"yb{hh}")
                          for hh in range(2)]
                    nb = len(PROC_B) - 1
                    for ui, (s, kind) in enumerate(PROC_B):
                        n = SLOTN[s]
                        ws = (8 - n) * P
                        if kind == 't':
                            a, b = ws + 512, 1024     # tail window
                        else:
                            a, b = ws, 1024           # pure window
                        nw = b - a
                        ksl = slice(s * P, (s + 1) * P)
                        # the two heads' S MMs run concurrently (row-tiled);
                        # concurrent writes into one PSUM bank fault the HW,
                        # so head1 always lands in bank 1 (col 512).
                        span = span_p.tile([P, 1024], f32, tag="span")
                        nc.tensor.matmul(span[:, 0:nw], kt[0:64, ksl],
                                         qt[0:64, a:b], start=True, stop=True)
                        nc.tensor.matmul(span[:, 512:512 + nw], kt[64:128, ksl],
                                         qt[64:128, a:b], start=True, stop=True)
                        pt = ptB_pool.tile([P, 1024], bf16, tag="ptB")
                        sview = span[:].rearrange("p (h x) -> p h x", h=2)[:, :, 0:nw]
                        pview = pt[:, 0:2 * nw].rearrange("p (h x) -> p h x", h=2)
                        nc.scalar.activation(pview, sview, AFT.Exp,
                                             bias=bz_sb[:], scale=0.125)
                        if kind == 'p':
                            apply_mask(pt, s, nw)
                        # AV into yb cols [a-512 : b-512]
                        for hh in range(2):
                            nc.tensor.matmul(
                                yb[hh][:, a - 512:512],
                                v_sb[g][s][:, off + hh * 65:off + hh * 65 + 65],
                                pt[:, hh * nw:hh * nw + nw],
                                start=(ui == 0), stop=(ui == nb))
                        # sliver: qb1 part of retained front subslot of s
                        if kind == 't' and ws > 0:
                            pa = ptA[s]
                            for hh in range(2):
                                nc.tensor.matmul(
                                    yb[hh][:, 0:ws],
                                    v_sb[g][s][:, off + hh * 65:off + hh * 65 + 65],
                                    pa[:, hh * 512 + 512 - ws:hh * 512 + 512],
                                    start=False, stop=False)
                        pump()
                    for hh, yy in ((0, yb[0]), (1, yb[1])):
                        with nc.allow_low_precision(reason="softmax recip"):
                            nc.vector.reciprocal(recs[2 + hh][:], yy[64:65, :])
                        nc.vector.tensor_copy(yT[p][hh * 64:(hh + 1) * 64, 512:1024],
                                              yy[0:64, :])

                    while pend:
                        pend.pop(0)()
                    pend_norm = norm_units(p, recs)
                for u in pend_norm:
                    u()

            # ---------------- output projection --------------------------
            with ExitStack() as ctx:
                wp_pool = ctx.enter_context(tc.tile_pool(name="wpj", bufs=8))
                wpj_sb = [wp_pool.tile([P, C], bf16, tag="wpj", name=f"wpj{i}")
                          for i in range(8)]
                for c in range(8):
                    nc.sync.dma_start(wpj_sb[c][:], wpj[c * P:(c + 1) * P, :])
                bp_pool = ctx.enter_context(tc.tile_pool(name="bpj", bufs=1))
                bpj_sb = bp_pool.tile([P, C], f32, tag="bpj")
                nc.sync.dma_start(bpj_sb[:], bpj)

                pj_p = ctx.enter_context(tc.tile_pool(name="pj", bufs=4, space="PSUM"))
                ost = ctx.enter_context(tc.tile_pool(name="ost", bufs=3))
                for tt in range(8):
                    ot = ost.tile([P, C], f32, tag="ost")
                    for co in range(2):
                        acc = pj_p.tile([P, 512], f32, tag="pj")
                        for c in range(8):
                            nc.tensor.matmul(acc[:], yT[c][:, tt * P:(tt + 1) * P],
                                             wpj_sb[c][:, co * 512:(co + 1) * 512],
                                             start=(c == 0), stop=(c == 7))
                        nc.vector.tensor_add(ot[:, co * 512:(co + 1) * 512], acc[:],
                                             bpj_sb[:, co * 512:(co + 1) * 512])
                    nc.sync.dma_start(outd[tt * P:(tt + 1) * P, :], ot[:])

    nc.compile()
    return nc


_NC_CACHE = None


def _get_program():
    global _NC_CACHE
    if _NC_CACHE is None:
        _NC_CACHE = _build_program()
    return _NC_CACHE


def _host_inputs(x, W_attn, b_attn, W_proj, b_proj):
    """Build the 8 per-core input maps."""
    import ml_dtypes
    bf = ml_dtypes.bfloat16
    x = np.asarray(x, dtype=np.float32)
    W_attn = np.asarray(W_attn, dtype=np.float32)
    b_attn = np.asarray(b_attn, dtype=np.float32)
    W_proj = np.asarray(W_proj, dtype=np.float32)
    b_proj = np.asarray(b_proj, dtype=np.float32)

    wqk = np.ascontiguousarray(W_attn[:, :2 * C]).astype(bf)
    bqk = np.empty((P, 16), np.float32)
    for dt in range(16):
        bqk[:, dt] = b_attn[dt * P:(dt + 1) * P]
    # V' weights: per head 64 V columns + one zero column (ones come via bias)
    wvp = np.zeros((C, VPW), np.float32)
    bvp_row = np.zeros(VPW, np.float32)
    for h in range(NH):
        wvp[:, h * 65:h * 65 + 64] = W_attn[:, 2 * C + h * HS:2 * C + (h + 1) * HS]
        bvp_row[h * 65:h * 65 + 64] = b_attn[2 * C + h * HS:2 * C + (h + 1) * HS]
        bvp_row[h * 65 + 64] = 1.0
    wvp = wvp.astype(bf)
    bvp = np.tile(bvp_row, (P, 1))
    bpj = np.tile(b_proj, (P, 1))
    wpj = W_proj.astype(bf)

    # static diagonal mask: tri[k, q] = 1 if k <= q (S^T layout)
    kk = np.arange(P)[:, None]
    qq = np.arange(P)[None, :]
    tri = (kk <= qq).astype(np.float32).astype(bf)

    in_maps = []
    for core in range(NCORES):
        b, cls = core // 2, core % 2
        own = list(range(cls, 16, 2))
        other = list(range(1 - cls, 16, 2))
        pi = own + other
        tok = np.concatenate([np.arange(t * P, (t + 1) * P) for t in pi])
        xkT = np.ascontiguousarray(x[b][tok].T).astype(bf)
        # slots 8-15 front-tile kill: acausal for class E (other tile > front
        # own tile), causal for class O
        km = np.empty((8, P, P), np.float32)
        for r in range(8):
            km[r] = 0.0 if other[r] > own[8 - SLOTN[8 + r]] else 1.0
        km = km.astype(bf)
        in_maps.append({
            "xkT": xkT, "wqk": wqk, "bqk": bqk, "wvp": wvp, "bvp": bvp,
            "wproj": wpj, "bproj": bpj, "tri": tri, "kmask": km,
            "bz": np.zeros((P, 1), np.float32),
        })
    return in_maps


def run(inputs, trace=False, tmpdir=None):
    from concourse.bass_utils import run_bass_kernel_spmd
    nc = _get_program()
    in_maps = _host_inputs(**inputs)
    res = run_bass_kernel_spmd(nc, in_maps, core_ids=list(range(NCORES)),
                               trace=trace, tmpdir=tmpdir)
    out = np.empty((B, T, C), np.float32)
    for core in range(NCORES):
        b, cls = core // 2, core % 2
        o = res.results[core]["out"]
        for j, t in enumerate(range(cls, 16, 2)):
            out[b, t * P:(t + 1) * P] = o[j * P:(j + 1) * P]
    return out, res


def kernel(x, W_attn, b_attn, W_proj, b_proj):
    out, _ = run(dict(x=x, W_attn=W_attn, b_attn=b_attn,
                      W_proj=W_proj, b_proj=b_proj))
    return out
